# revision 10
# baseline (speedup 1.0000x reference)
"""Trainium2 Bass kernel for nn_LogitGenerator.

Computes, for x:[T,768], label:[T], W_proj:[256,768], b_proj:[256],
label_embeddings:[504,256]:
    xp   = x @ W_proj.T + b_proj
    xn   = xp / max(||xp||, 1e-8)
    en   = emb / max(||emb||, 1e-8)
    sims = (xn @ en.T) / 0.1
    pos  = sims[t, label[t]]
    negs = where(dup_mask, -inf, sims)
    out  = [pos | negs]  : [T, 505]

Sharding: data-parallel over T across 8 cores (4096 tokens each); weights,
normalized-embedding matrix and the mask/pos lookup table are replicated.
"""

import math
import os

import numpy as np

# ---- problem constants (hardcoded per harness contract) ----
T = 32768
D = 768          # encoder dim (contraction of phase A)
F = 256          # final dim
C = 504          # num classes
NCORES = 8
TC = T // NCORES  # 4096 tokens per core
P = 128
KC = D // P       # 6 k-chunks
FC = F // P       # 2 f-chunks
TPB = 4           # 128-token tiles per 512-token block
NBLK = TC // (P * TPB)  # 8 blocks per core
EPS = 1e-8
INV_TEMP = 10.0   # 1 / 0.1

_cache: dict = {}


def _build_nc():
    import concourse.bass as bass
    import concourse.mybir as mybir
    import concourse.tile as tile
    from concourse import bacc
    from concourse.masks import make_identity

    f32 = mybir.dt.float32
    bf16 = mybir.dt.bfloat16
    i32 = mybir.dt.int32
    Act = mybir.ActivationFunctionType
    Alu = mybir.AluOpType
    # matmul operand dtype for phases A/B: bf16 streams 1 col/cycle on the PE
    # vs 2 for fp32 (accumulation is fp32 in PSUM either way)
    mmdt = bf16 if os.environ.get("KERNEL_MM_DTYPE", "bf16") == "bf16" else f32

    nc = bacc.Bacc(
        "TRN2", target_bir_lowering=False, debug=False,
        enable_asserts=False, num_devices=NCORES,
    )

    x_d = nc.dram_tensor("x", [TC, D], f32, kind="ExternalInput").ap()
    labw_d = nc.dram_tensor("labw", [P, TC // P], i32, kind="ExternalInput").ap()
    wkf_d = nc.dram_tensor("wkf", [P, KC, F], mmdt, kind="ExternalInput").ap()
    ent_d = nc.dram_tensor("ent", [P, FC, C], mmdt, kind="ExternalInput").ap()
    bf_d = nc.dram_tensor("bf", [P, FC], f32, kind="ExternalInput").ap()
    tab_d = nc.dram_tensor("tab", [C, 1024], bf16, kind="ExternalInput").ap()
    out_d = nc.dram_tensor("out", [TC, C + 1], f32, kind="ExternalOutput").ap()

    x_t = x_d.rearrange("(n p) k -> p n k", p=P)        # [128, 32, 768]
    out_t = out_d.rearrange("(n p) c -> p n c", p=P)    # [128, 32, 505]

    with tile.TileContext(nc) as tc:
        with (
            tc.tile_pool(name="const", bufs=1) as cpool,
            tc.tile_pool(name="xin", bufs=2) as xpool,
            tc.tile_pool(name="xt", bufs=2) as xtpool,
            tc.tile_pool(name="xp", bufs=2) as xppool,
            tc.tile_pool(name="xp2", bufs=2) as xp2pool,
            tc.tile_pool(name="rr", bufs=2) as rpool,
            tc.tile_pool(name="gath", bufs=4) as gpool,
            tc.tile_pool(name="sout", bufs=4) as spool,
            tc.tile_pool(name="scrap", bufs=2) as tpool,
            tc.tile_pool(name="ps_xt", bufs=2, space="PSUM") as xtpsum,
            tc.tile_pool(name="ps_xp", bufs=2, space="PSUM") as xppsum,
            tc.tile_pool(name="ps_n", bufs=1, space="PSUM") as npsum,
            tc.tile_pool(name="ps_b", bufs=3, space="PSUM") as bpsum,
        ):
            # ---- constants ----
            wkf = cpool.tile([P, KC, F], mmdt)
            nc.sync.dma_start(wkf[:], wkf_d)
            ent = cpool.tile([P, FC, C], mmdt)
            nc.sync.dma_start(ent[:], ent_d)
            bfb = cpool.tile([P, FC], f32)
            nc.sync.dma_start(bfb[:], bf_d)
            labw = cpool.tile([P, TC // P], i32)
            nc.sync.dma_start(labw[:], labw_d)
            ident = cpool.tile([P, P], f32)
            make_identity(nc, ident[:])
            ones = cpool.tile([P, 1], f32)
            nc.gpsimd.memset(ones[:], 1.0)

            for b in range(NBLK):
                # ---- load 512 tokens ----
                x_sb = xpool.tile([P, TPB, D], f32)
                nc.sync.dma_start(x_sb[:], x_t[:, b * TPB:(b + 1) * TPB, :])

                # ---- transpose x -> xT chunks [k=128p, t=512] ----
                xt_sb = xtpool.tile([P, KC, TPB * P], mmdt)
                for ck in range(KC):
                    xt_ps = xtpsum.tile([P, TPB * P], f32)
                    for j in range(TPB):
                        nc.tensor.transpose(
                            xt_ps[:, j * P:(j + 1) * P],
                            x_sb[:, j, ck * P:(ck + 1) * P],
                            ident[:],
                        )
                    nc.scalar.copy(xt_sb[:, ck, :], xt_ps[:])

                # ---- phase A: xpT[f, t] = W @ x^T  (+bias later) ----
                xpT = xppool.tile([P, FC, TPB * P], mmdt)
                xpT2 = xp2pool.tile([P, FC, TPB * P], f32)
                for fc in range(FC):
                    xp_ps = xppsum.tile([P, TPB * P], f32)
                    for ck in range(KC):
                        nc.tensor.matmul(
                            xp_ps[:],
                            lhsT=wkf[:, ck, fc * P:(fc + 1) * P],
                            rhs=xt_sb[:, ck, :],
                            start=(ck == 0),
                            stop=(ck == KC - 1),
                        )
                    # xpT = psum + b  (per-partition bias add, DVE)
                    nc.vector.tensor_scalar_add(
                        xpT[:, fc, :], xp_ps[:], bfb[:, fc:fc + 1])
                    # xpT2 = (psum + b)^2  (ACT)
                    nc.scalar.activation(
                        xpT2[:, fc, :], xp_ps[:], Act.Square,
                        bias=bfb[:, fc:fc + 1], scale=1.0)

                # ---- row norms: n2[t] = sum_f xpT2[f, t] via ones-matmul ----
                n2_ps = npsum.tile([P, TPB], f32)
                for j in range(TPB):
                    for fc in range(FC):
                        nc.tensor.matmul(
                            n2_ps[:, j:j + 1],
                            lhsT=xpT2[:, fc, j * P:(j + 1) * P],
                            rhs=ones[:],
                            start=(fc == 0),
                            stop=(fc == FC - 1),
                        )
                nrm = rpool.tile([P, TPB], f32, tag="nrm")
                nc.scalar.activation(nrm[:], n2_ps[:], Act.Sqrt)
                nrmc = rpool.tile([P, TPB], f32, tag="nrmc")
                nc.vector.tensor_scalar_max(nrmc[:], nrm[:], EPS)
                rec = rpool.tile([P, TPB], f32, tag="rec")
                nc.vector.reciprocal(rec[:], nrmc[:])
                r10 = rpool.tile([P, TPB], f32, tag="r10")
                nc.vector.tensor_scalar_mul(r10[:], rec[:], INV_TEMP)

                # ---- per 128-token tile: sims matmul + epilogue ----
                for j in range(TPB):
                    t_idx = b * TPB + j
                    import concourse.bass as _b
                    g_sb = gpool.tile([P, 1024], bf16)
                    nc.gpsimd.indirect_dma_start(
                        out=g_sb[:],
                        out_offset=None,
                        in_=tab_d,
                        in_offset=_b.IndirectOffsetOnAxis(
                            ap=labw[:, t_idx:t_idx + 1], axis=0),
                    )
                    s_ps = bpsum.tile([P, C], f32)
                    for fc in range(FC):
                        nc.tensor.matmul(
                            s_ps[:],
                            lhsT=xpT[:, fc, j * P:(j + 1) * P],
                            rhs=ent[:, fc, :],
                            start=(fc == 0),
                            stop=(fc == FC - 1),
                        )
                    s_sb = spool.tile([P, 512], f32)
                    # negs = (sims * r10[t]) min maskrow   (maskrow: -inf dup / +inf)
                    nc.vector.scalar_tensor_tensor(
                        out=s_sb[:, 1:C + 1],
                        in0=s_ps[:],
                        scalar=r10[:, j:j + 1],
                        in1=g_sb[:, 0:C],
                        op0=Alu.mult,
                        op1=Alu.min,
                    )
                    # pos_raw[t] = sum_c(sims * onehotrow): exact (other terms 0.0)
                    trash = tpool.tile([P, C], f32, tag="trash")
                    posr = tpool.tile([P, 1], f32, tag="posr")
                    nc.vector.scalar_tensor_tensor(
                        out=trash[:],
                        in0=s_ps[:],
                        scalar=1.0,
                        in1=g_sb[:, 512:512 + C],
                        op0=Alu.mult,
                        op1=Alu.mult,
                        accum_out=posr[:],
                    )
                    nc.vector.tensor_scalar_mul(
                        s_sb[:, 0:1], posr[:], r10[:, j:j + 1])
                    nc.sync.dma_start(out_t[:, t_idx, :], s_sb[:, 0:C + 1])

    nc.compile()
    return nc


def _host_prep(x, label, W_proj, b_proj, label_embeddings):
    import ml_dtypes

    x = np.ascontiguousarray(np.asarray(x, dtype=np.float32))
    label = np.asarray(label).astype(np.int32).reshape(-1)
    W = np.asarray(W_proj, dtype=np.float32)
    bvec = np.asarray(b_proj, dtype=np.float32)
    emb = np.asarray(label_embeddings, dtype=np.float32)

    # replicated tensors
    mmdt = (ml_dtypes.bfloat16
            if os.environ.get("KERNEL_MM_DTYPE", "bf16") == "bf16"
            else np.float32)
    wkf = np.ascontiguousarray(
        W.T.reshape(KC, P, F).transpose(1, 0, 2)).astype(mmdt)  # [128, 6, 256]
    bfh = np.ascontiguousarray(bvec.reshape(FC, P).T)           # [128, 2]
    nrm = np.maximum(np.linalg.norm(emb, axis=1, keepdims=True), EPS)
    en = emb / nrm
    ent = np.ascontiguousarray(
        en.T.reshape(FC, P, C).transpose(1, 0, 2)).astype(mmdt)  # [128, 2, 504]

    # duplicate-row groups (mask row c' = -inf where emb[c] == emb[c'])
    _, grp = np.unique(emb.view(np.uint32), axis=0, return_inverse=True)
    eq = grp[:, None] == grp[None, :]                           # [C, C]
    tab = np.zeros((C, 1024), dtype=ml_dtypes.bfloat16)
    minmask = np.where(eq, -np.inf, np.inf).astype(ml_dtypes.bfloat16)
    posmask = np.where(np.eye(C, dtype=bool), 1.0,
                       0.0).astype(ml_dtypes.bfloat16)
    tab[:, 0:C] = minmask
    tab[:, 512:512 + C] = posmask

    in_maps = []
    for c in range(NCORES):
        xs = x[c * TC:(c + 1) * TC]
        ls = label[c * TC:(c + 1) * TC]
        labww = np.ascontiguousarray(ls.reshape(TC // P, P).T)  # [128, 32]
        in_maps.append({
            "x": xs, "labw": labww, "wkf": wkf, "ent": ent,
            "bf": bfh, "tab": tab,
        })
    return in_maps


def kernel(x, label, W_proj, b_proj, label_embeddings):
    from concourse import bass_utils

    if "nc" not in _cache:
        _cache["nc"] = _build_nc()
    nc = _cache["nc"]

    in_maps = _host_prep(x, label, W_proj, b_proj, label_embeddings)
    trace = bool(int(os.environ.get("KERNEL_TRACE", "0")))
    res = bass_utils.run_bass_kernel_spmd(
        nc, in_maps, core_ids=list(range(NCORES)), trace=trace)
    _cache["last_result"] = res
    out = np.concatenate([r["out"] for r in res.results], axis=0)
    return out


# revision 17
# speedup vs baseline: 1.0465x; 1.0465x over previous
"""Trainium2 Bass kernel for nn_LogitGenerator.

Computes, for x:[T,768], label:[T], W_proj:[256,768], b_proj:[256],
label_embeddings:[504,256]:
    xp   = x @ W_proj.T + b_proj
    xn   = xp / max(||xp||, 1e-8)
    en   = emb / max(||emb||, 1e-8)
    sims = (xn @ en.T) / 0.1
    pos  = sims[t, label[t]]
    negs = where(dup_mask, -inf, sims)
    out  = [pos | negs]  : [T, 505]

Sharding: data-parallel over T across 8 cores (4096 tokens each); weights,
normalized-embedding matrix and the mask/pos lookup table are replicated.
"""

import math
import os

import numpy as np

# ---- problem constants (hardcoded per harness contract) ----
T = 32768
D = 768          # encoder dim (contraction of phase A)
F = 256          # final dim
C = 504          # num classes
NCORES = 8
TC = T // NCORES  # 4096 tokens per core
P = 128
KC = D // P       # 6 k-chunks
FC = F // P       # 2 f-chunks
TPB = 4           # 128-token tiles per 512-token block
NBLK = TC // (P * TPB)  # 8 blocks per core
EPS = 1e-8
INV_TEMP = 10.0   # 1 / 0.1

_cache: dict = {}


def _build_nc():
    import concourse.bass as bass
    import concourse.mybir as mybir
    import concourse.tile as tile
    from concourse import bacc
    from concourse.masks import make_identity

    f32 = mybir.dt.float32
    bf16 = mybir.dt.bfloat16
    i32 = mybir.dt.int32
    Act = mybir.ActivationFunctionType
    Alu = mybir.AluOpType
    # matmul operand dtype for phases A/B: bf16 streams 1 col/cycle on the PE
    # vs 2 for fp32 (accumulation is fp32 in PSUM either way)
    mmdt = bf16 if os.environ.get("KERNEL_MM_DTYPE", "bf16") == "bf16" else f32
    # compute the pos-onehot row on DVE (iota==label) instead of gathering it:
    # halves the mask-table DMA traffic
    oh_dve = os.environ.get("KERNEL_POS_ONEHOT", "0") == "1"
    tabw = 512 if oh_dve else 1024

    nc = bacc.Bacc(
        "TRN2", target_bir_lowering=False, debug=False,
        enable_asserts=False, num_devices=NCORES,
    )

    x_d = nc.dram_tensor("x", [TC, D], f32, kind="ExternalInput").ap()
    labw_d = nc.dram_tensor("labw", [P, TC // P], i32, kind="ExternalInput").ap()
    wkf_d = nc.dram_tensor("wkf", [P, KC, F], mmdt, kind="ExternalInput").ap()
    ent_d = nc.dram_tensor("ent", [P, FC, C], mmdt, kind="ExternalInput").ap()
    bf_d = nc.dram_tensor("bf", [P, FC], f32, kind="ExternalInput").ap()
    tab_d = nc.dram_tensor("tab", [C, tabw], bf16, kind="ExternalInput").ap()
    if oh_dve:
        iotab_d = nc.dram_tensor("iotab", [P, C], f32, kind="ExternalInput").ap()
        lblf_d = nc.dram_tensor("lblf", [P, TC // P], f32,
                                kind="ExternalInput").ap()
    out_d = nc.dram_tensor("out", [TC, C + 1], f32, kind="ExternalOutput").ap()

    x_t = x_d.rearrange("(n p) k -> p n k", p=P)        # [128, 32, 768]
    out_t = out_d.rearrange("(n p) c -> p n c", p=P)    # [128, 32, 505]

    with tile.TileContext(nc) as tc:
        with (
            tc.tile_pool(name="const", bufs=1) as cpool,
            tc.tile_pool(name="xin", bufs=2) as xpool,
            tc.tile_pool(name="xt", bufs=2) as xtpool,
            tc.tile_pool(name="xp", bufs=2) as xppool,
            tc.tile_pool(name="xp2", bufs=2) as xp2pool,
            tc.tile_pool(name="rr", bufs=2) as rpool,
            tc.tile_pool(name="gath", bufs=4) as gpool,
            tc.tile_pool(name="sout", bufs=4) as spool,
            tc.tile_pool(name="scrap", bufs=2) as tpool,
            tc.tile_pool(name="ps_xt", bufs=2, space="PSUM") as xtpsum,
            tc.tile_pool(name="ps_xp", bufs=2, space="PSUM") as xppsum,
            tc.tile_pool(name="ps_n", bufs=1, space="PSUM") as npsum,
            tc.tile_pool(name="ps_b", bufs=3, space="PSUM") as bpsum,
        ):
            # ---- constants ----
            wkf = cpool.tile([P, KC, F], mmdt)
            nc.sync.dma_start(wkf[:], wkf_d)
            ent = cpool.tile([P, FC, C], mmdt)
            nc.sync.dma_start(ent[:], ent_d)
            bfb = cpool.tile([P, FC], f32)
            nc.sync.dma_start(bfb[:], bf_d)
            labw = cpool.tile([P, TC // P], i32)
            nc.sync.dma_start(labw[:], labw_d)
            ident = cpool.tile([P, P], f32)
            make_identity(nc, ident[:])
            ones = cpool.tile([P, 1], f32)
            nc.gpsimd.memset(ones[:], 1.0)
            if oh_dve:
                iotab = cpool.tile([P, C], f32)
                nc.sync.dma_start(iotab[:], iotab_d)
                lblf = cpool.tile([P, TC // P], f32)
                nc.sync.dma_start(lblf[:], lblf_d)

            for b in range(NBLK):
                # ---- load 512 tokens ----
                x_sb = xpool.tile([P, TPB, D], f32)
                nc.sync.dma_start(x_sb[:], x_t[:, b * TPB:(b + 1) * TPB, :])

                # ---- transpose x -> xT chunks [k=128p, t=512] ----
                xt_sb = xtpool.tile([P, KC, TPB * P], mmdt)
                for ck in range(KC):
                    xt_ps = xtpsum.tile([P, TPB * P], f32)
                    for j in range(TPB):
                        nc.tensor.transpose(
                            xt_ps[:, j * P:(j + 1) * P],
                            x_sb[:, j, ck * P:(ck + 1) * P],
                            ident[:],
                        )
                    nc.scalar.copy(xt_sb[:, ck, :], xt_ps[:])

                # ---- phase A: xpT[f, t] = W @ x^T  (+bias later) ----
                xpT = xppool.tile([P, FC, TPB * P], mmdt)
                xpT2 = xp2pool.tile([P, FC, TPB * P], f32)
                for fc in range(FC):
                    xp_ps = xppsum.tile([P, TPB * P], f32)
                    for ck in range(KC):
                        nc.tensor.matmul(
                            xp_ps[:],
                            lhsT=wkf[:, ck, fc * P:(fc + 1) * P],
                            rhs=xt_sb[:, ck, :],
                            start=(ck == 0),
                            stop=(ck == KC - 1),
                        )
                    # xpT = psum + b  (per-partition bias add, DVE)
                    nc.vector.tensor_scalar_add(
                        xpT[:, fc, :], xp_ps[:], bfb[:, fc:fc + 1])
                    # xpT2 = (psum + b)^2  (ACT)
                    nc.scalar.activation(
                        xpT2[:, fc, :], xp_ps[:], Act.Square,
                        bias=bfb[:, fc:fc + 1], scale=1.0)

                # ---- row norms: n2[t] = sum_f xpT2[f, t] via ones-matmul ----
                n2_ps = npsum.tile([P, TPB], f32)
                for j in range(TPB):
                    for fc in range(FC):
                        nc.tensor.matmul(
                            n2_ps[:, j:j + 1],
                            lhsT=xpT2[:, fc, j * P:(j + 1) * P],
                            rhs=ones[:],
                            start=(fc == 0),
                            stop=(fc == FC - 1),
                        )
                nrm = rpool.tile([P, TPB], f32, tag="nrm")
                nc.scalar.activation(nrm[:], n2_ps[:], Act.Sqrt)
                nrmc = rpool.tile([P, TPB], f32, tag="nrmc")
                nc.vector.tensor_scalar_max(nrmc[:], nrm[:], EPS)
                rec = rpool.tile([P, TPB], f32, tag="rec")
                nc.vector.reciprocal(rec[:], nrmc[:])
                r10 = rpool.tile([P, TPB], f32, tag="r10")
                nc.vector.tensor_scalar_mul(r10[:], rec[:], INV_TEMP)

                # ---- per 128-token tile: sims matmul + epilogue ----
                for j in range(TPB):
                    t_idx = b * TPB + j
                    import concourse.bass as _b
                    g_sb = gpool.tile([P, tabw], bf16)
                    nc.gpsimd.indirect_dma_start(
                        out=g_sb[:],
                        out_offset=None,
                        in_=tab_d,
                        in_offset=_b.IndirectOffsetOnAxis(
                            ap=labw[:, t_idx:t_idx + 1], axis=0),
                    )
                    s_ps = bpsum.tile([P, C], f32)
                    for fc in range(FC):
                        nc.tensor.matmul(
                            s_ps[:],
                            lhsT=xpT[:, fc, j * P:(j + 1) * P],
                            rhs=ent[:, fc, :],
                            start=(fc == 0),
                            stop=(fc == FC - 1),
                        )
                    s_sb = spool.tile([P, 512], f32)
                    # negs = (sims * r10[t]) min maskrow   (maskrow: -inf dup / +inf)
                    nc.vector.scalar_tensor_tensor(
                        out=s_sb[:, 1:C + 1],
                        in0=s_ps[:],
                        scalar=r10[:, j:j + 1],
                        in1=g_sb[:, 0:C],
                        op0=Alu.mult,
                        op1=Alu.min,
                    )
                    # pos_raw[t] = sum_c(sims * onehotrow): exact (other terms 0.0)
                    trash = tpool.tile([P, C], f32, tag="trash")
                    posr = tpool.tile([P, 1], f32, tag="posr")
                    if oh_dve:
                        oh_sb = tpool.tile([P, C], f32, tag="oh")
                        nc.vector.tensor_scalar(
                            out=oh_sb[:], in0=iotab[:],
                            scalar1=lblf[:, t_idx:t_idx + 1], scalar2=None,
                            op0=Alu.is_equal)
                        posrow = oh_sb[:]
                    else:
                        posrow = g_sb[:, 512:512 + C]
                    nc.vector.scalar_tensor_tensor(
                        out=trash[:],
                        in0=s_ps[:],
                        scalar=1.0,
                        in1=posrow,
                        op0=Alu.mult,
                        op1=Alu.mult,
                        accum_out=posr[:],
                    )
                    nc.vector.tensor_scalar_mul(
                        s_sb[:, 0:1], posr[:], r10[:, j:j + 1])
                    nc.sync.dma_start(out_t[:, t_idx, :], s_sb[:, 0:C + 1])

    nc.compile()
    return nc


def _host_prep(x, label, W_proj, b_proj, label_embeddings):
    import ml_dtypes

    x = np.ascontiguousarray(np.asarray(x, dtype=np.float32))
    label = np.asarray(label).astype(np.int32).reshape(-1)
    W = np.asarray(W_proj, dtype=np.float32)
    bvec = np.asarray(b_proj, dtype=np.float32)
    emb = np.asarray(label_embeddings, dtype=np.float32)

    # replicated tensors
    mmdt = (ml_dtypes.bfloat16
            if os.environ.get("KERNEL_MM_DTYPE", "bf16") == "bf16"
            else np.float32)
    wkf = np.ascontiguousarray(
        W.T.reshape(KC, P, F).transpose(1, 0, 2)).astype(mmdt)  # [128, 6, 256]
    bfh = np.ascontiguousarray(bvec.reshape(FC, P).T)           # [128, 2]
    nrm = np.maximum(np.linalg.norm(emb, axis=1, keepdims=True), EPS)
    en = emb / nrm
    ent = np.ascontiguousarray(
        en.T.reshape(FC, P, C).transpose(1, 0, 2)).astype(mmdt)  # [128, 2, 504]

    # duplicate-row groups (mask row c' = -inf where emb[c] == emb[c'])
    _, grp = np.unique(emb.view(np.uint32), axis=0, return_inverse=True)
    eq = grp[:, None] == grp[None, :]                           # [C, C]
    oh_dve = os.environ.get("KERNEL_POS_ONEHOT", "0") == "1"
    tabw = 512 if oh_dve else 1024
    tab = np.zeros((C, tabw), dtype=ml_dtypes.bfloat16)
    minmask = np.where(eq, -np.inf, np.inf).astype(ml_dtypes.bfloat16)
    tab[:, 0:C] = minmask
    if not oh_dve:
        posmask = np.where(np.eye(C, dtype=bool), 1.0,
                           0.0).astype(ml_dtypes.bfloat16)
        tab[:, 512:512 + C] = posmask
    iotab = np.broadcast_to(np.arange(C, dtype=np.float32), (P, C)).copy()

    in_maps = []
    for c in range(NCORES):
        xs = x[c * TC:(c + 1) * TC]
        ls = label[c * TC:(c + 1) * TC]
        labww = np.ascontiguousarray(ls.reshape(TC // P, P).T)  # [128, 32]
        im = {
            "x": xs, "labw": labww, "wkf": wkf, "ent": ent,
            "bf": bfh, "tab": tab,
        }
        if oh_dve:
            im["iotab"] = iotab
            im["lblf"] = labww.astype(np.float32)
        in_maps.append(im)
    return in_maps


def kernel(x, label, W_proj, b_proj, label_embeddings):
    from concourse import bass_utils

    if "nc" not in _cache:
        _cache["nc"] = _build_nc()
    nc = _cache["nc"]

    in_maps = _host_prep(x, label, W_proj, b_proj, label_embeddings)
    trace = bool(int(os.environ.get("KERNEL_TRACE", "0")))
    res = bass_utils.run_bass_kernel_spmd(
        nc, in_maps, core_ids=list(range(NCORES)), trace=trace)
    _cache["last_result"] = res
    out = np.concatenate([r["out"] for r in res.results], axis=0)
    return out
